# revision 27
# baseline (speedup 1.0000x reference)
"""Trainium2 Bass kernel for the gated two-path (semantic+RoPE-geometric) causal
attention layer.  8-core sharding: 2 batches x 4 head-groups (4 heads each).

Reference computation (B=2, S=2048, D_MODEL=2048, H=16, DS=DG=64, DV=128):
  qs=x@wq_sem, ks=x@wk_sem, qg=rope(x@wq_geo), kg=rope(x@wk_geo), v=x@wv
  scores = g*qs.ks/8 + (1-g)*qg.kg/8 ; causal softmax ; out=(attn@v)@wo

Per-core kernel strategy (scores/AV/out-proj matmuls float32r, projection
inputs bf16, transposed dataflow):
  - host folds sigmoid(gate)/sqrt(d) scales into wq and concatenates
    [sem|geo] per head so each head's QK^T is one K=128 contraction
  - weights/activations are pre-tiled on the host into p-major SBUF layouts
    so each logical block moves as ONE large DMA with >=1KB-contiguous runs
    (HWDGE descriptor cost ~0.6us each dominates at small sizes)
  - projections: qcatT/kcatT [128, S] per head via lhsT=weight tiles,
    rhs=xT chunks; v in natural [token, dv] layout via lhsT=xT tiles;
    rope applied per 512-token slice right after each eviction
  - scores^T [k,128 x q,512] per k-tile; causal mask added via an
    identity-lhsT matmul of a sliding window into one [128,896] staircase
  - exp on ScalarE (no max subtraction; |scores| <~ 8 << 88); AV +
    ones-matmul denominators accumulate in PSUM, gpsimd broadcast + fast
    reciprocal; normalization fused into the PSUM->SBUF eviction mul
  - output projection fused per 512-token chunk right after its attention
    (wo resident in SBUF, loaded into the space phase 1 frees); host
    transposes and sums the 4 head-group partials per batch
"""

import os
import sys

sys.path.insert(0, "/opt/trn_rl_repo")

import ml_dtypes
import numpy as np

import concourse.mybir as mybir
import concourse.tile as tile
from concourse import bacc, bass_isa
from concourse.bass_utils import run_bass_kernel_spmd

F32 = mybir.dt.float32
F32R = mybir.dt.float32r
F16 = mybir.dt.float16
BF16 = mybir.dt.bfloat16

# bf16 projections: halves projection DMA traffic; rel err ~3.3e-3 vs ~3.4e-4
PROJ_BF16 = os.environ.get("KERNEL_PROJ_BF16", "1") == "1"
PROJ_DT = BF16 if PROJ_BF16 else F32R
PROJ_NP = ml_dtypes.bfloat16 if PROJ_BF16 else np.float32

B, S, DM = 2, 2048, 2048
H, DS, DG, DV = 16, 64, 64, 128
HPC = 4                      # heads per core
NCORES = 8
DH = DS + DG                 # 128, concat [sem|geo] per head
NKT = S // 128               # 16 key tiles
NQB = S // 512               # 4 query blocks
NTCH = 4                     # token chunks of 512
NDMK = DM // 128             # 16 contraction tiles
MASK_VAL = -10000.0

_CACHED = {}


def _build(repeat=1):
    nc = bacc.Bacc("TRN2", target_bir_lowering=False, debug=False,
                   num_devices=NCORES)

    # p-major layouts (see _host_prep): one big DMA per logical block
    xT = nc.dram_tensor("xT", [128, NDMK, S], PROJ_DT,
                        kind="ExternalInput").ap()
    wqk_d = nc.dram_tensor("wqk", [2 * HPC, 128, NDMK, 128], PROJ_DT,
                           kind="ExternalInput").ap()
    wv_d = nc.dram_tensor("wv", [128, NDMK, 512], PROJ_DT,
                          kind="ExternalInput").ap()
    wo_d = nc.dram_tensor("wo", [128, NDMK, HPC, 128], F32R,
                          kind="ExternalInput").ap()
    cos2_d = nc.dram_tensor("cos2", [DG, S], F16, kind="ExternalInput").ap()
    sins_d = nc.dram_tensor("sins", [DG, S], F16, kind="ExternalInput").ap()
    masks_d = nc.dram_tensor("masks", [128, 896], F32R,
                             kind="ExternalInput").ap()
    ident_d = nc.dram_tensor("ident", [128, 128], F32R,
                             kind="ExternalInput").ap()
    out_d = nc.dram_tensor("out", [128, NDMK, S], F32,
                           kind="ExternalOutput").ap()

    Exp = mybir.ActivationFunctionType.Exp

    with tile.TileContext(nc) as tc:
      for _rep in range(repeat):
        with tc.tile_pool(name="consts", bufs=1) as cpool:
            # trig tables at base partition 64 so two-input DVE rope ops
            # share their operands' base partition
            trig = cpool.tile([128, 2, S], F16)
            masksB = cpool.tile([128, 896], F32R)
            ident = cpool.tile([128, 128], F32R)

            with tc.tile_pool(name="persist", bufs=1) as ppool:
                qcatT = ppool.tile([128, HPC, S], F32R)
                kcatT = ppool.tile([128, HPC, S], F32R)
                v_sb = ppool.tile([128, NKT, 512], F32R)

                # ---------------- phase 1: projections (+ rope fused) ------
                with tc.tile_pool(name="xt", bufs=2) as xtp, \
                     tc.tile_pool(name="wcol", bufs=10) as wcp, \
                     tc.tile_pool(name="wvst", bufs=1) as wvp, \
                     tc.tile_pool(name="rot", bufs=2) as rpool, \
                     tc.tile_pool(name="psA", bufs=4, space="PSUM") as psA:

                    wv_res = wvp.tile([128, NDMK, 512], PROJ_DT)
                    wcs = {}

                    def load_wq(fb):
                        wc_q = wcp.tile([128, NDMK, 128], PROJ_DT, tag="wc")
                        nc.sync.dma_start(out=wc_q[:], in_=wqk_d[fb, :, :, :])
                        wcs[fb] = wc_q
                        return wc_q

                    for tch in range(NTCH):
                        ts_ = slice(tch * 512, tch * 512 + 512)
                        # first feature column's weights before the x chunk
                        # so the first PSUM chain can start immediately
                        if tch == 0:
                            wc0 = wcp.tile([128, NDMK, 128], PROJ_DT,
                                           tag="wc")
                            nc.sync.dma_start(out=wc0[:, 0:4, :],
                                              in_=wqk_d[0, :, 0:4, :])
                            wcs[0] = wc0
                        xts = xtp.tile([128, NDMK, 512], PROJ_DT, tag="xt")
                        if tch == 0:
                            # split the cold-start loads so the first
                            # matmul chain starts after ~1/4 of the data
                            nc.sync.dma_start(out=xts[:, 0:4, :],
                                              in_=xT[:, 0:4, ts_])
                            nc.sync.dma_start(out=wc0[:, 4:16, :],
                                              in_=wqk_d[0, :, 4:16, :])
                            nc.sync.dma_start(out=xts[:, 4:16, :],
                                              in_=xT[:, 4:16, ts_])
                        else:
                            nc.sync.dma_start(out=xts[:], in_=xT[:, :, ts_])
                        if tch == 0:
                            # weight blocks for the next chains first, then
                            # consts (needed ~7us in), wv last (needed ~30us)
                            for fb_pre in (1, 2, 3):
                                load_wq(fb_pre)
                            nc.sync.dma_start(out=trig[64:128, 0, :],
                                              in_=cos2_d[:, :])
                            nc.sync.dma_start(out=trig[64:128, 1, :],
                                              in_=sins_d[:, :])
                            nc.sync.dma_start(out=masksB[:],
                                              in_=masks_d[:, :])
                            nc.sync.dma_start(out=ident[:], in_=ident_d[:, :])
                            nc.sync.dma_start(out=wv_res[:], in_=wv_d[:, :, :])
                        # qcat / kcat columns: 8 feature tiles of 128
                        for fb in range(2 * HPC):
                            h = fb % HPC
                            wc = wcs.get(fb)
                            if wc is None:
                                wc = load_wq(fb)
                            ps_t = psA.tile([128, 512], F32, tag="ps")
                            for dmk in range(NDMK):
                                nc.tensor.matmul(
                                    ps_t[:],
                                    wc[:, dmk, :],
                                    xts[:, dmk, :],
                                    start=(dmk == 0), stop=(dmk == NDMK - 1))
                            X = qcatT if fb < HPC else kcatT
                            nc.scalar.copy(X[:, h, ts_], ps_t[:])
                            # rope this 512-token slice of the geo half
                            rot = rpool.tile([128, 512], F32R, tag="rot")
                            nc.gpsimd.tensor_copy(rot[64:96, :],
                                                  X[96:128, h, ts_])
                            nc.gpsimd.tensor_copy(rot[96:128, :],
                                                  X[64:96, h, ts_])
                            nc.vector.tensor_mul(rot[64:128, :],
                                                 rot[64:128, :],
                                                 trig[64:128, 1, ts_])
                            nc.vector.tensor_mul(X[64:128, h, ts_],
                                                 X[64:128, h, ts_],
                                                 trig[64:128, 0, ts_])
                            nc.vector.tensor_add(X[64:128, h, ts_],
                                                 X[64:128, h, ts_],
                                                 rot[64:128, :])
                        # v: natural layout, 4 token sub-tiles
                        for tsub in range(4):
                            tt = tch * 4 + tsub
                            ps_v = psA.tile([128, 512], F32, tag="ps")
                            for dmk in range(NDMK):
                                nc.tensor.matmul(
                                    ps_v[:],
                                    xts[:, dmk, tsub * 128:tsub * 128 + 128],
                                    wv_res[:, dmk, :],
                                    start=(dmk == 0), stop=(dmk == NDMK - 1))
                            nc.scalar.copy(v_sb[:, tt, :], ps_v[:])

                # wo lands in the SBUF space phase 1 frees; one contiguous
                # DMA queued behind the phase-1 loads
                with tc.tile_pool(name="wop", bufs=1) as wop:
                    wo_sb = wop.tile([128, NDMK, HPC, 128], F32R)
                    nc.sync.dma_start(out=wo_sb[:], in_=wo_d[:, :, :, :])

                    # ------- phase 2: attention + fused output projection --
                    with tc.tile_pool(name="es", bufs=4) as espool, \
                         tc.tile_pool(name="acc", bufs=4) as accpool, \
                         tc.tile_pool(name="bc", bufs=3) as bcpool, \
                         tc.tile_pool(name="ao", bufs=2) as aopool, \
                         tc.tile_pool(name="ost", bufs=2) as ostp, \
                         tc.tile_pool(name="psS", bufs=2,
                                      space="PSUM") as psS, \
                         tc.tile_pool(name="psO", bufs=2,
                                      space="PSUM") as psO, \
                         tc.tile_pool(name="psW", bufs=2,
                                      space="PSUM") as psW:
                        def attn_head(J, h, attn_o):
                            qs_ = slice(J * 512, J * 512 + 512)
                            nkt = 4 * J + 4          # causal k-tiles
                            ngrp = nkt // 2
                            if True:
                                ps_o = psO.tile([128, 512], F32, tag="po")
                                # two independent DVE accumulation chains
                                # for the softmax denominator
                                accs = [accpool.tile([128, 512], F32,
                                                     tag="acc",
                                                     name=f"acc{J}_{h}_{i}")
                                        for i in range(min(2, ngrp))]
                                for g in range(ngrp):
                                    ps_sc = psS.tile([128, 1024], F32,
                                                     tag="sc")
                                    es = espool.tile([128, 1024], F32R,
                                                     tag="es")
                                    for t2 in range(2):
                                        kt = 2 * g + t2
                                        sl = slice(t2 * 512, t2 * 512 + 512)
                                        diag = kt >= 4 * J
                                        if not diag:
                                            nc.tensor.matmul(
                                                ps_sc[:, sl],
                                                kcatT[:, h, kt * 128:
                                                      kt * 128 + 128],
                                                qcatT[:, h, qs_],
                                                start=True, stop=True)
                                            continue
                                        # diagonal: the full-width staircase
                                        # matmul (start=True) doubles as the
                                        # -1e4 init for the q<kp region the
                                        # narrowed scores matmul skips
                                        t = kt - 4 * J
                                        q0 = 128 * t
                                        j0 = 384 - q0
                                        nc.tensor.matmul(
                                            ps_sc[:, sl], ident[:],
                                            masksB[:, j0:j0 + 512],
                                            start=True, stop=False)
                                        nc.tensor.matmul(
                                            ps_sc[:, t2 * 512 + q0:
                                                  t2 * 512 + 512],
                                            kcatT[:, h, kt * 128:
                                                  kt * 128 + 128],
                                            qcatT[:, h, J * 512 + q0:
                                                  J * 512 + 512],
                                            start=False, stop=True)
                                    nc.scalar.activation(es[:], ps_sc[:],
                                                         Exp)
                                    acc = accs[g % len(accs)]
                                    if g < 2:
                                        nc.vector.tensor_add(
                                            acc[:], es[:, 0:512],
                                            es[:, 512:1024])
                                    else:
                                        nc.vector.tensor_add(
                                            acc[:], acc[:], es[:, 0:512])
                                        nc.vector.tensor_add(
                                            acc[:], acc[:], es[:, 512:1024])
                                    for t2 in range(2):
                                        kt = 2 * g + t2
                                        q0 = max(0, 128 * (kt - 4 * J))
                                        nc.tensor.matmul(
                                            ps_o[:, q0:512],
                                            v_sb[:, kt,
                                                 h * 128:h * 128 + 128],
                                            es[:, t2 * 512 + q0:
                                                t2 * 512 + 512],
                                            start=(kt == 0),
                                            stop=(kt == nkt - 1))
                                # normalize: join chains, partition-sum on
                                # gpsimd, fast reciprocal, scale the
                                # PSUM->SBUF eviction
                                if len(accs) == 2:
                                    accf = accpool.tile([128, 512], F32,
                                                        tag="acc")
                                    nc.vector.tensor_add(accf[:], accs[0][:],
                                                         accs[1][:])
                                else:
                                    accf = accs[0]
                                red = bcpool.tile([128, 512], F32, tag="red")
                                nc.gpsimd.partition_all_reduce(
                                    red[:], accf[:], 128,
                                    bass_isa.ReduceOp.add)
                                bcr = bcpool.tile([128, 512], F32, tag="bcr")
                                nc.vector.reciprocal_approx_fast(bcr[:],
                                                                 red[:])
                                nc.vector.tensor_mul(attn_o[:, h, :],
                                                     ps_o[:], bcr[:])

                        def oproj(J, attn_o):
                            # fused output projection for this token chunk;
                            # out DMAs batched 2 feature tiles at a time
                            qs_ = slice(J * 512, J * 512 + 512)
                            gsz = 2
                            for dgrp in range(NDMK // gsz):
                                o4 = ostp.tile([128, gsz, 512], F32,
                                               tag=f"ost{gsz}")
                                for di in range(gsz):
                                    dmt = dgrp * gsz + di
                                    ps_w = psW.tile([128, 512], F32,
                                                    tag="pw")
                                    for h in range(HPC):
                                        nc.tensor.matmul(
                                            ps_w[:],
                                            wo_sb[:, dmt, h, :],
                                            attn_o[:, h, :],
                                            start=(h == 0),
                                            stop=(h == HPC - 1))
                                    if di % 2 == 0:
                                        nc.scalar.copy(o4[:, di, :], ps_w[:])
                                    else:
                                        nc.vector.tensor_copy(o4[:, di, :],
                                                              ps_w[:])
                                nc.sync.dma_start(
                                    out=out_d[:, dgrp * gsz:
                                              dgrp * gsz + gsz, qs_],
                                    in_=o4[:])

                        # emit chunk J's out-projection after chunk J+1's
                        # first head: PE has score/AV work queued while the
                        # last head's DVE norm chain drains
                        pending = None
                        for J in range(NQB):
                            attn_o = aopool.tile([128, HPC, 512], F32R,
                                                 tag="ao")
                            for h in range(HPC):
                                attn_head(J, h, attn_o)
                                if h == 0 and pending is not None:
                                    oproj(*pending)
                                    pending = None
                            pending = (J, attn_o)
                        oproj(*pending)

    nc.compile()
    return nc


def _host_prep(x, wq_sem, wk_sem, wq_geo, wk_geo, wv, wo, gate_logit):
    """Build the 8 per-core input maps."""
    g = 1.0 / (1.0 + np.exp(-gate_logit.astype(np.float64)))  # [H]
    sc = 1.0 / np.sqrt(DS)

    half = DG // 2
    inv_freq = 1.0 / (10000.0 ** (np.arange(half, dtype=np.float64) / half))
    ang = np.arange(S, dtype=np.float64)[:, None] * inv_freq[None, :]  # [S, 32]
    cosT = np.cos(ang).T
    sinT = np.sin(ang).T
    cos2 = np.ascontiguousarray(
        np.concatenate([cosT, cosT], 0).astype(np.float16))          # [64, S]
    sins = np.ascontiguousarray(
        np.concatenate([-sinT, sinT], 0).astype(np.float16))         # [64, S]

    # sliding causal staircase: masks[kp, j] = 0 iff (j - 384) >= kp.
    # diag variant t uses window [384-128t : 896-128t].
    kp = np.arange(128)[:, None]
    j = np.arange(896)[None, :]
    masks = np.where(j - 384 >= kp, 0.0, MASK_VAL).astype(np.float32)
    ident = np.eye(128, dtype=np.float32)

    in_maps = []
    for c in range(NCORES):
        b, hg = divmod(c, HPC)
        heads = range(hg * HPC, hg * HPC + HPC)
        wq_cat = np.empty((DM, HPC * DH), dtype=np.float32)
        wk_cat = np.empty((DM, HPC * DH), dtype=np.float32)
        for i, h in enumerate(heads):
            gh = g[h]
            wq_cat[:, i * DH:i * DH + DS] = \
                wq_sem[:, h * DS:(h + 1) * DS] * np.float32(gh * sc)
            wq_cat[:, i * DH + DS:(i + 1) * DH] = \
                wq_geo[:, h * DG:(h + 1) * DG] * np.float32((1.0 - gh) * sc)
            wk_cat[:, i * DH:i * DH + DS] = wk_sem[:, h * DS:(h + 1) * DS]
            wk_cat[:, i * DH + DS:(i + 1) * DH] = wk_geo[:, h * DG:(h + 1) * DG]
        # pre-tile: wqk[fb, p, dmk, c] = w_cat[dmk*128+p, fb*128+c]
        wq_t = wq_cat.reshape(NDMK, 128, HPC, 128).transpose(2, 1, 0, 3)
        wk_t = wk_cat.reshape(NDMK, 128, HPC, 128).transpose(2, 1, 0, 3)
        wqk = np.ascontiguousarray(np.concatenate([wq_t, wk_t], 0))
        h0 = hg * HPC * DV
        wv_slice = wv[:, h0:h0 + HPC * DV]
        wv_t = np.ascontiguousarray(
            wv_slice.reshape(NDMK, 128, HPC * DV).transpose(1, 0, 2))
        wo_slice = wo[h0:h0 + HPC * DV, :]
        # wo[p, dmt, h, c] = wo_slice[h*128+p, dmt*128+c]  (p-major)
        wo_t = np.ascontiguousarray(
            wo_slice.reshape(HPC, 128, NDMK, 128).transpose(1, 2, 0, 3))
        # xT[p, dmk, s] = x[b].T[dmk*128+p, s]  (p-major)
        xTt = np.ascontiguousarray(
            x[b].T.reshape(NDMK, 128, S).transpose(1, 0, 2))
        in_maps.append({
            "xT": xTt.astype(PROJ_NP),
            "wqk": wqk.astype(PROJ_NP),
            "wv": wv_t.astype(PROJ_NP),
            "wo": wo_t,
            "cos2": cos2,
            "sins": sins,
            "masks": masks,
            "ident": ident,
        })
    return in_maps


def _run(in_maps, **kw):
    if "nc" not in _CACHED:
        _CACHED["nc"] = _build()
    return run_bass_kernel_spmd(_CACHED["nc"], in_maps,
                                core_ids=list(range(NCORES)), **kw)


def _gather_out(o):
    """[128, NDMK, S] p-major partial -> [S, DM]."""
    return np.asarray(o).transpose(1, 0, 2).reshape(DM, S).T


def kernel(x, wq_sem, wk_sem, wq_geo, wk_geo, wv, wo, gate_logit, **_kw):
    x = np.asarray(x, dtype=np.float32)
    wq_sem = np.asarray(wq_sem, dtype=np.float32)
    wk_sem = np.asarray(wk_sem, dtype=np.float32)
    wq_geo = np.asarray(wq_geo, dtype=np.float32)
    wk_geo = np.asarray(wk_geo, dtype=np.float32)
    wv = np.asarray(wv, dtype=np.float32)
    wo = np.asarray(wo, dtype=np.float32)
    gate_logit = np.asarray(gate_logit, dtype=np.float32)

    in_maps = _host_prep(x, wq_sem, wk_sem, wq_geo, wk_geo, wv, wo, gate_logit)
    res = _run(in_maps)
    out = np.zeros((B, S, DM), dtype=np.float32)
    for c in range(NCORES):
        out[c // HPC] += _gather_out(res.results[c]["out"])
    return out


# revision 30
# speedup vs baseline: 6.2374x; 6.2374x over previous
"""Trainium2 Bass kernel for the gated two-path (semantic+RoPE-geometric) causal
attention layer.  8-core sharding: 2 batches x 4 head-groups (4 heads each).

Reference computation (B=2, S=2048, D_MODEL=2048, H=16, DS=DG=64, DV=128):
  qs=x@wq_sem, ks=x@wk_sem, qg=rope(x@wq_geo), kg=rope(x@wk_geo), v=x@wv
  scores = g*qs.ks/8 + (1-g)*qg.kg/8 ; causal softmax ; out=(attn@v)@wo

Per-core kernel strategy (scores/AV/out-proj matmuls float32r, projection
inputs bf16, transposed dataflow):
  - host folds sigmoid(gate)/sqrt(d) scales into wq and concatenates
    [sem|geo] per head so each head's QK^T is one K=128 contraction
  - weights/activations are pre-tiled on the host into p-major SBUF layouts
    so each logical block moves as ONE large DMA with >=1KB-contiguous runs
    (HWDGE descriptor cost ~0.6us each dominates at small sizes)
  - projections: qcatT/kcatT [128, S] per head via lhsT=weight tiles,
    rhs=xT chunks; v in natural [token, dv] layout via lhsT=xT tiles;
    rope applied per 512-token slice right after each eviction
  - scores^T [k,128 x q,512] per k-tile; causal mask added via an
    identity-lhsT matmul of a sliding window into one [128,896] staircase
  - exp on ScalarE (no max subtraction; |scores| <~ 8 << 88); AV +
    ones-matmul denominators accumulate in PSUM, gpsimd broadcast + fast
    reciprocal; normalization fused into the PSUM->SBUF eviction mul
  - output projection fused per 512-token chunk right after its attention
    (wo resident in SBUF, loaded into the space phase 1 frees); host
    transposes and sums the 4 head-group partials per batch
"""

import os
import sys

sys.path.insert(0, "/opt/trn_rl_repo")

import ml_dtypes
import numpy as np

import concourse.mybir as mybir
import concourse.tile as tile
from concourse import bacc, bass_isa
from concourse.bass_utils import run_bass_kernel_spmd

F32 = mybir.dt.float32
F32R = mybir.dt.float32r
F16 = mybir.dt.float16
BF16 = mybir.dt.bfloat16

# bf16 projections: halves projection DMA traffic; rel err ~3.3e-3 vs ~3.4e-4
PROJ_BF16 = os.environ.get("KERNEL_PROJ_BF16", "1") == "1"
PROJ_DT = BF16 if PROJ_BF16 else F32R
PROJ_NP = ml_dtypes.bfloat16 if PROJ_BF16 else np.float32

B, S, DM = 2, 2048, 2048
H, DS, DG, DV = 16, 64, 64, 128
HPC = 4                      # heads per core
NCORES = 8
DH = DS + DG                 # 128, concat [sem|geo] per head
NKT = S // 128               # 16 key tiles
NQB = S // 512               # 4 query blocks
NTCH = 4                     # token chunks of 512
NDMK = DM // 128             # 16 contraction tiles
MASK_VAL = -10000.0

_CACHED = {}


def _build(repeat=1):
    nc = bacc.Bacc("TRN2", target_bir_lowering=False, debug=False,
                   num_devices=NCORES)

    # p-major layouts (see _host_prep): one big DMA per logical block
    xT = nc.dram_tensor("xT", [128, NDMK, S], PROJ_DT,
                        kind="ExternalInput").ap()
    wqk_d = nc.dram_tensor("wqk", [2 * HPC, 128, NDMK, 128], PROJ_DT,
                           kind="ExternalInput").ap()
    wv_d = nc.dram_tensor("wv", [128, NDMK, 512], PROJ_DT,
                          kind="ExternalInput").ap()
    wo_d = nc.dram_tensor("wo", [128, NDMK, HPC, 128], BF16,
                          kind="ExternalInput").ap()
    cos2_d = nc.dram_tensor("cos2", [DG, S], F16, kind="ExternalInput").ap()
    sins_d = nc.dram_tensor("sins", [DG, S], F16, kind="ExternalInput").ap()
    masks_d = nc.dram_tensor("masks", [128, 896], F32R,
                             kind="ExternalInput").ap()
    ident_d = nc.dram_tensor("ident", [128, 128], F32R,
                             kind="ExternalInput").ap()
    out_d = nc.dram_tensor("out", [128, NDMK, S], F32,
                           kind="ExternalOutput").ap()

    Exp = mybir.ActivationFunctionType.Exp

    with tile.TileContext(nc) as tc:
      for _rep in range(repeat):
        with tc.tile_pool(name="consts", bufs=1) as cpool:
            # trig tables at base partition 64 so two-input DVE rope ops
            # share their operands' base partition
            trig = cpool.tile([128, 2, S], F16)
            masksB = cpool.tile([128, 896], F32R)
            ident = cpool.tile([128, 128], F32R)

            with tc.tile_pool(name="persist", bufs=1) as ppool:
                qcatT = ppool.tile([128, HPC, S], F32R)
                kcatT = ppool.tile([128, HPC, S], F32R)
                v_sb = ppool.tile([128, NKT, 512], F32R)

                # ---------------- phase 1: projections (+ rope fused) ------
                with tc.tile_pool(name="xt", bufs=2) as xtp, \
                     tc.tile_pool(name="wcol", bufs=10) as wcp, \
                     tc.tile_pool(name="wvst", bufs=1) as wvp, \
                     tc.tile_pool(name="rot", bufs=2) as rpool, \
                     tc.tile_pool(name="psA", bufs=4, space="PSUM") as psA:

                    wv_res = wvp.tile([128, NDMK, 512], PROJ_DT)
                    wcs = {}

                    def load_wq(fb):
                        wc_q = wcp.tile([128, NDMK, 128], PROJ_DT, tag="wc")
                        nc.sync.dma_start(out=wc_q[:], in_=wqk_d[fb, :, :, :])
                        wcs[fb] = wc_q
                        return wc_q

                    for tch in range(NTCH):
                        ts_ = slice(tch * 512, tch * 512 + 512)
                        # first feature column's weights before the x chunk
                        # so the first PSUM chain can start immediately
                        if tch == 0:
                            wc0 = wcp.tile([128, NDMK, 128], PROJ_DT,
                                           tag="wc")
                            nc.sync.dma_start(out=wc0[:, 0:4, :],
                                              in_=wqk_d[0, :, 0:4, :])
                            wcs[0] = wc0
                        xts = xtp.tile([128, NDMK, 512], PROJ_DT, tag="xt")
                        if tch == 0:
                            # split the cold-start loads so the first
                            # matmul chain starts after ~1/4 of the data
                            nc.sync.dma_start(out=xts[:, 0:4, :],
                                              in_=xT[:, 0:4, ts_])
                            nc.sync.dma_start(out=wc0[:, 4:16, :],
                                              in_=wqk_d[0, :, 4:16, :])
                            nc.sync.dma_start(out=xts[:, 4:16, :],
                                              in_=xT[:, 4:16, ts_])
                        else:
                            nc.sync.dma_start(out=xts[:], in_=xT[:, :, ts_])
                        if tch == 0:
                            # weight blocks for the next chains first, then
                            # consts (needed ~7us in), wv last (needed ~30us)
                            for fb_pre in (1, 2, 3):
                                load_wq(fb_pre)
                            nc.sync.dma_start(out=trig[64:128, 0, :],
                                              in_=cos2_d[:, :])
                            nc.sync.dma_start(out=trig[64:128, 1, :],
                                              in_=sins_d[:, :])
                            nc.sync.dma_start(out=masksB[:],
                                              in_=masks_d[:, :])
                            nc.sync.dma_start(out=ident[:], in_=ident_d[:, :])
                            nc.sync.dma_start(out=wv_res[:], in_=wv_d[:, :, :])
                        # qcat / kcat columns: 8 feature tiles of 128
                        for fb in range(2 * HPC):
                            h = fb % HPC
                            wc = wcs.get(fb)
                            if wc is None:
                                wc = load_wq(fb)
                            ps_t = psA.tile([128, 512], F32, tag="ps")
                            for dmk in range(NDMK):
                                nc.tensor.matmul(
                                    ps_t[:],
                                    wc[:, dmk, :],
                                    xts[:, dmk, :],
                                    start=(dmk == 0), stop=(dmk == NDMK - 1))
                            X = qcatT if fb < HPC else kcatT
                            nc.scalar.copy(X[:, h, ts_], ps_t[:])
                            # rope this 512-token slice of the geo half
                            rot = rpool.tile([128, 512], F32R, tag="rot")
                            nc.gpsimd.tensor_copy(rot[64:96, :],
                                                  X[96:128, h, ts_])
                            nc.gpsimd.tensor_copy(rot[96:128, :],
                                                  X[64:96, h, ts_])
                            nc.vector.tensor_mul(rot[64:128, :],
                                                 rot[64:128, :],
                                                 trig[64:128, 1, ts_])
                            nc.vector.tensor_mul(X[64:128, h, ts_],
                                                 X[64:128, h, ts_],
                                                 trig[64:128, 0, ts_])
                            nc.vector.tensor_add(X[64:128, h, ts_],
                                                 X[64:128, h, ts_],
                                                 rot[64:128, :])
                        # v: natural layout, 4 token sub-tiles
                        for tsub in range(4):
                            tt = tch * 4 + tsub
                            ps_v = psA.tile([128, 512], F32, tag="ps")
                            for dmk in range(NDMK):
                                nc.tensor.matmul(
                                    ps_v[:],
                                    xts[:, dmk, tsub * 128:tsub * 128 + 128],
                                    wv_res[:, dmk, :],
                                    start=(dmk == 0), stop=(dmk == NDMK - 1))
                            nc.scalar.copy(v_sb[:, tt, :], ps_v[:])

                # wo lands in the SBUF space phase 1 frees; one contiguous
                # DMA queued behind the phase-1 loads
                with tc.tile_pool(name="wop", bufs=1) as wop:
                    wo_sb = wop.tile([128, NDMK, HPC, 128], BF16)
                    nc.sync.dma_start(out=wo_sb[:], in_=wo_d[:, :, :, :])

                    # ------- phase 2: attention + fused output projection --
                    with tc.tile_pool(name="es", bufs=3) as espool, \
                         tc.tile_pool(name="acc", bufs=4) as accpool, \
                         tc.tile_pool(name="bc", bufs=2) as bcpool, \
                         tc.tile_pool(name="ao", bufs=4) as aopool, \
                         tc.tile_pool(name="ost", bufs=2) as ostp, \
                         tc.tile_pool(name="psS", bufs=2,
                                      space="PSUM") as psS, \
                         tc.tile_pool(name="psO", bufs=2,
                                      space="PSUM") as psO, \
                         tc.tile_pool(name="psW", bufs=2,
                                      space="PSUM") as psW:
                        def attn_head(J, h, attn_o):
                            qs_ = slice(J * 512, J * 512 + 512)
                            nkt = 4 * J + 4          # causal k-tiles
                            ngrp = nkt // 2
                            if True:
                                ps_o = psO.tile([128, 512], F32, tag="po")
                                # two independent DVE accumulation chains
                                # for the softmax denominator
                                accs = [accpool.tile([128, 512], F32,
                                                     tag="acc",
                                                     name=f"acc{J}_{h}_{i}")
                                        for i in range(min(2, ngrp))]
                                for g in range(ngrp):
                                    ps_sc = psS.tile([128, 1024], F32,
                                                     tag="sc")
                                    es = espool.tile([128, 1024], F32R,
                                                     tag="es")
                                    for t2 in range(2):
                                        kt = 2 * g + t2
                                        sl = slice(t2 * 512, t2 * 512 + 512)
                                        diag = kt >= 4 * J
                                        if not diag:
                                            nc.tensor.matmul(
                                                ps_sc[:, sl],
                                                kcatT[:, h, kt * 128:
                                                      kt * 128 + 128],
                                                qcatT[:, h, qs_],
                                                start=True, stop=True)
                                            continue
                                        # diagonal: the full-width staircase
                                        # matmul (start=True) doubles as the
                                        # -1e4 init for the q<kp region the
                                        # narrowed scores matmul skips
                                        t = kt - 4 * J
                                        q0 = 128 * t
                                        j0 = 384 - q0
                                        nc.tensor.matmul(
                                            ps_sc[:, sl], ident[:],
                                            masksB[:, j0:j0 + 512],
                                            start=True, stop=False)
                                        nc.tensor.matmul(
                                            ps_sc[:, t2 * 512 + q0:
                                                  t2 * 512 + 512],
                                            kcatT[:, h, kt * 128:
                                                  kt * 128 + 128],
                                            qcatT[:, h, J * 512 + q0:
                                                  J * 512 + 512],
                                            start=False, stop=True)
                                    nc.scalar.activation(es[:], ps_sc[:],
                                                         Exp)
                                    acc = accs[g % len(accs)]
                                    if g < 2:
                                        nc.vector.tensor_add(
                                            acc[:], es[:, 0:512],
                                            es[:, 512:1024])
                                    else:
                                        nc.vector.tensor_add(
                                            acc[:], acc[:], es[:, 0:512])
                                        nc.vector.tensor_add(
                                            acc[:], acc[:], es[:, 512:1024])
                                    for t2 in range(2):
                                        kt = 2 * g + t2
                                        q0 = max(0, 128 * (kt - 4 * J))
                                        nc.tensor.matmul(
                                            ps_o[:, q0:512],
                                            v_sb[:, kt,
                                                 h * 128:h * 128 + 128],
                                            es[:, t2 * 512 + q0:
                                                t2 * 512 + 512],
                                            start=(kt == 0),
                                            stop=(kt == nkt - 1))
                                # normalize: join chains, partition-sum on
                                # gpsimd, fast reciprocal, scale the
                                # PSUM->SBUF eviction
                                if len(accs) == 2:
                                    accf = accpool.tile([128, 512], F32,
                                                        tag="acc")
                                    nc.vector.tensor_add(accf[:], accs[0][:],
                                                         accs[1][:])
                                else:
                                    accf = accs[0]
                                red = bcpool.tile([128, 512], F32, tag="red")
                                nc.gpsimd.partition_all_reduce(
                                    red[:], accf[:], 128,
                                    bass_isa.ReduceOp.add)
                                bcr = bcpool.tile([128, 512], F32, tag="bcr")
                                nc.vector.reciprocal_approx_fast(bcr[:],
                                                                 red[:])
                                nc.vector.tensor_mul(attn_o[:, h, :],
                                                     ps_o[:], bcr[:])

                        def oproj_piece(J, attn_o, dgrp):
                            # out-projection slice: 2 feature tiles + 1 DMA
                            qs_ = slice(J * 512, J * 512 + 512)
                            gsz = 2
                            o4 = ostp.tile([128, gsz, 512], F32,
                                           tag=f"ost{gsz}")
                            for di in range(gsz):
                                dmt = dgrp * gsz + di
                                ps_w = psW.tile([128, 512], F32, tag="pw")
                                for h in range(HPC):
                                    nc.tensor.matmul(
                                        ps_w[:],
                                        wo_sb[:, dmt, h, :],
                                        attn_o[:, h, :],
                                        start=(h == 0),
                                        stop=(h == HPC - 1))
                                if di % 2 == 0:
                                    nc.scalar.copy(o4[:, di, :], ps_w[:])
                                else:
                                    nc.vector.tensor_copy(o4[:, di, :],
                                                          ps_w[:])
                            nc.sync.dma_start(
                                out=out_d[:, dgrp * gsz:dgrp * gsz + gsz,
                                          qs_],
                                in_=o4[:])

                        # interleave DVE/ACT-heavy late chunks with light
                        # early ones so no engine saturates a section, and
                        # spread out-projection pieces as PE filler between
                        # attention units (also hides the per-head norm
                        # chain latency)
                        unit_order = [(3, 0), (0, 0), (3, 1), (0, 1),
                                      (3, 2), (0, 2), (3, 3), (0, 3),
                                      (2, 0), (1, 0), (2, 1), (1, 1),
                                      (2, 2), (1, 2), (2, 3), (1, 3)]
                        attn_os = {}
                        pieces = []
                        for J, h in unit_order:
                            if J not in attn_os:
                                attn_os[J] = aopool.tile(
                                    [128, HPC, 512], BF16, tag="ao",
                                    name=f"attn_o{J}")
                            attn_head(J, h, attn_os[J])
                            if h == HPC - 1:
                                pieces.extend(
                                    (J, attn_os[J], dgrp)
                                    for dgrp in range(NDMK // 2))
                            for _ in range(2):
                                if pieces:
                                    oproj_piece(*pieces.pop(0))
                        while pieces:
                            oproj_piece(*pieces.pop(0))

    nc.compile()
    return nc


def _host_prep(x, wq_sem, wk_sem, wq_geo, wk_geo, wv, wo, gate_logit):
    """Build the 8 per-core input maps."""
    g = 1.0 / (1.0 + np.exp(-gate_logit.astype(np.float64)))  # [H]
    sc = 1.0 / np.sqrt(DS)

    half = DG // 2
    inv_freq = 1.0 / (10000.0 ** (np.arange(half, dtype=np.float64) / half))
    ang = np.arange(S, dtype=np.float64)[:, None] * inv_freq[None, :]  # [S, 32]
    cosT = np.cos(ang).T
    sinT = np.sin(ang).T
    cos2 = np.ascontiguousarray(
        np.concatenate([cosT, cosT], 0).astype(np.float16))          # [64, S]
    sins = np.ascontiguousarray(
        np.concatenate([-sinT, sinT], 0).astype(np.float16))         # [64, S]

    # sliding causal staircase: masks[kp, j] = 0 iff (j - 384) >= kp.
    # diag variant t uses window [384-128t : 896-128t].
    kp = np.arange(128)[:, None]
    j = np.arange(896)[None, :]
    masks = np.where(j - 384 >= kp, 0.0, MASK_VAL).astype(np.float32)
    ident = np.eye(128, dtype=np.float32)

    in_maps = []
    for c in range(NCORES):
        b, hg = divmod(c, HPC)
        heads = range(hg * HPC, hg * HPC + HPC)
        wq_cat = np.empty((DM, HPC * DH), dtype=np.float32)
        wk_cat = np.empty((DM, HPC * DH), dtype=np.float32)
        for i, h in enumerate(heads):
            gh = g[h]
            wq_cat[:, i * DH:i * DH + DS] = \
                wq_sem[:, h * DS:(h + 1) * DS] * np.float32(gh * sc)
            wq_cat[:, i * DH + DS:(i + 1) * DH] = \
                wq_geo[:, h * DG:(h + 1) * DG] * np.float32((1.0 - gh) * sc)
            wk_cat[:, i * DH:i * DH + DS] = wk_sem[:, h * DS:(h + 1) * DS]
            wk_cat[:, i * DH + DS:(i + 1) * DH] = wk_geo[:, h * DG:(h + 1) * DG]
        # pre-tile: wqk[fb, p, dmk, c] = w_cat[dmk*128+p, fb*128+c]
        wq_t = wq_cat.reshape(NDMK, 128, HPC, 128).transpose(2, 1, 0, 3)
        wk_t = wk_cat.reshape(NDMK, 128, HPC, 128).transpose(2, 1, 0, 3)
        wqk = np.ascontiguousarray(np.concatenate([wq_t, wk_t], 0))
        h0 = hg * HPC * DV
        wv_slice = wv[:, h0:h0 + HPC * DV]
        wv_t = np.ascontiguousarray(
            wv_slice.reshape(NDMK, 128, HPC * DV).transpose(1, 0, 2))
        wo_slice = wo[h0:h0 + HPC * DV, :]
        # wo[p, dmt, h, c] = wo_slice[h*128+p, dmt*128+c]  (p-major)
        wo_t = np.ascontiguousarray(
            wo_slice.reshape(HPC, 128, NDMK, 128).transpose(1, 2, 0, 3)
        ).astype(ml_dtypes.bfloat16)
        # xT[p, dmk, s] = x[b].T[dmk*128+p, s]  (p-major)
        xTt = np.ascontiguousarray(
            x[b].T.reshape(NDMK, 128, S).transpose(1, 0, 2))
        in_maps.append({
            "xT": xTt.astype(PROJ_NP),
            "wqk": wqk.astype(PROJ_NP),
            "wv": wv_t.astype(PROJ_NP),
            "wo": wo_t,
            "cos2": cos2,
            "sins": sins,
            "masks": masks,
            "ident": ident,
        })
    return in_maps


def _run(in_maps, **kw):
    if "nc" not in _CACHED:
        _CACHED["nc"] = _build()
    return run_bass_kernel_spmd(_CACHED["nc"], in_maps,
                                core_ids=list(range(NCORES)), **kw)


def _gather_out(o):
    """[128, NDMK, S] p-major partial -> [S, DM]."""
    return np.asarray(o).transpose(1, 0, 2).reshape(DM, S).T


def kernel(x, wq_sem, wk_sem, wq_geo, wk_geo, wv, wo, gate_logit, **_kw):
    x = np.asarray(x, dtype=np.float32)
    wq_sem = np.asarray(wq_sem, dtype=np.float32)
    wk_sem = np.asarray(wk_sem, dtype=np.float32)
    wq_geo = np.asarray(wq_geo, dtype=np.float32)
    wk_geo = np.asarray(wk_geo, dtype=np.float32)
    wv = np.asarray(wv, dtype=np.float32)
    wo = np.asarray(wo, dtype=np.float32)
    gate_logit = np.asarray(gate_logit, dtype=np.float32)

    in_maps = _host_prep(x, wq_sem, wk_sem, wq_geo, wk_geo, wv, wo, gate_logit)
    res = _run(in_maps)
    out = np.zeros((B, S, DM), dtype=np.float32)
    for c in range(NCORES):
        out[c // HPC] += _gather_out(res.results[c]["out"])
    return out
